# revision 6
# baseline (speedup 1.0000x reference)
"""Trainium2 Bass kernel for the multimodal BERT fusion block.

Contract: kernel(**inputs) takes FULL unsharded numpy inputs (as produced by
setup_inputs()), runs an SPMD Bass kernel on 8 NeuronCores (data-parallel over
the batch dim, params replicated), and returns the FULL outputs
(h[:,0], text_att1, fusion_att1) as numpy arrays.

Math per batch b (S=512 tokens, H=768, P=30 proj dim, FD=74 audio feat):
  textT[b]  = Wt @ hidden[b]^T                    [30, 512]
  ssq       = sum(textT^2) over ALL batches       (global -> AllReduce)
  inv_w2    = ssq^-0.5   (w = ssq^0.25; text/w gram scale = 1/w^2)
  text_att1 = relu((textT^T textT) * inv_w2)      [512, 512]   (output)
  audT[b]   = Wa @ audio[b]^T                     [30, 512]
  audio_att = relu(audT^T audT)
  F_pre     = tw*text_att1 + aw*audio_att ; fusion_att1 = relu(F_pre+fb) (output)
  row0: softmax(F_pre[0,:] + am[b,:]) -> e/sum ;  fd0 = e@hidden[b]/sum + hidden[b,0]
  h0 = LN(fd0 @ dW^T + db) * ln_w + ln_b          [768]        (output h[:,0])
"""
import ctypes
import contextlib
import sys
import types

import numpy as np

import concourse.bass as bass
import concourse.bacc as bacc
import concourse.mybir as mybir
import concourse.tile as tile
from concourse import masks
from concourse.bass_utils import run_bass_kernel_spmd

B, S, H, FD, P = 64, 512, 768, 74, 30
N_CORES = 8
BPC = B // N_CORES          # 8 batches per core
HC, SC = H // 128, S // 128  # 6, 4
LN_EPS = 1e-12

F32 = mybir.dt.float32
F32R = mybir.dt.float32r
AF = mybir.ActivationFunctionType
ALU = mybir.AluOpType

USE_F32R = True
PROFILE = False          # set True (from test.py) to capture HW exec time
LAST_EXEC_NS = None

_compiled = {}


def _r(ap):
    """Reinterpret an f32 AP as float32r for fast (1 cyc/row) PE matmuls."""
    return ap.bitcast(F32R) if USE_F32R else ap


def _build(tw: float, aw: float, fb: float):
    nc = bacc.Bacc("TRN2", target_bir_lowering=False, debug=False,
                   num_devices=N_CORES)

    hid_d = nc.dram_tensor("hid", [BPC, S, H], F32, kind="ExternalInput").ap()
    aud_d = nc.dram_tensor("aud", [BPC, S, FD], F32, kind="ExternalInput").ap()
    am_d = nc.dram_tensor("am", [BPC, S], F32, kind="ExternalInput").ap()
    wt_d = nc.dram_tensor("wt", [P, H], F32, kind="ExternalInput").ap()
    wa_d = nc.dram_tensor("wa", [P, FD], F32, kind="ExternalInput").ap()
    dw_d = nc.dram_tensor("dw", [H, H], F32, kind="ExternalInput").ap()
    db_d = nc.dram_tensor("db", [H], F32, kind="ExternalInput").ap()
    lnw_d = nc.dram_tensor("lnw", [H], F32, kind="ExternalInput").ap()
    lnb_d = nc.dram_tensor("lnb", [H], F32, kind="ExternalInput").ap()

    tatt_d = nc.dram_tensor("t_att", [BPC, S, S], F32, kind="ExternalOutput").ap()
    fatt_d = nc.dram_tensor("f_att", [BPC, S, S], F32, kind="ExternalOutput").ap()
    out0_d = nc.dram_tensor("out0", [BPC, H], F32, kind="ExternalOutput").ap()

    with tile.TileContext(nc) as tc:
        _body(nc, tc, tw, aw, fb,
              hid_d, aud_d, am_d, wt_d, wa_d, dw_d, db_d, lnw_d, lnb_d,
              tatt_d, fatt_d, out0_d)
    nc.compile()
    return nc


def _body(nc, tc, tw, aw, fb,
          hid_d, aud_d, am_d, wt_d, wa_d, dw_d, db_d, lnw_d, lnb_d,
          tatt_d, fatt_d, out0_d):
    from contextlib import ExitStack
    ctx = ExitStack()
    with ctx:
        const = ctx.enter_context(tc.tile_pool(name="const", bufs=1))
        persist = ctx.enter_context(tc.tile_pool(name="persist", bufs=1))
        work = ctx.enter_context(tc.tile_pool(name="work", bufs=2))
        att = ctx.enter_context(tc.tile_pool(name="att", bufs=5))
        smalls = ctx.enter_context(tc.tile_pool(name="smalls", bufs=2))
        dram = ctx.enter_context(tc.tile_pool(name="dram", bufs=1, space="DRAM"))

        ps_tr = ctx.enter_context(tc.tile_pool(name="ps_tr", bufs=2, space="PSUM"))
        ps_mm = ctx.enter_context(tc.tile_pool(name="ps_mm", bufs=2, space="PSUM"))
        ps_g = ctx.enter_context(tc.tile_pool(name="ps_g", bufs=2, space="PSUM"))
        ps_x = ctx.enter_context(tc.tile_pool(name="ps_x", bufs=1, space="PSUM"))

        # ---------------- constants / params ----------------
        ident = const.tile([128, 128], F32)
        masks.make_identity(nc, ident[:])

        ones_p = const.tile([P, 1], F32)
        nc.gpsimd.memset(ones_p[:], 1.0)

        # Wt^T: [P, H] -> HC chunks [128, P]
        wt_nat = const.tile([P, H], F32)
        nc.sync.dma_start(wt_nat[:], wt_d[:])
        wtT = const.tile([128, HC, P], F32R)
        for c in range(HC):
            tp = ps_tr.tile([128, P], F32, tag="tr")
            nc.tensor.transpose(tp[:], wt_nat[:, c * 128:(c + 1) * 128],
                                ident[:P, :P])
            nc.scalar.copy(wtT[:, c, :], tp[:])

        # Wa^T: [P, FD] -> [FD, P]
        wa_nat = const.tile([P, FD], F32)
        nc.sync.dma_start(wa_nat[:], wa_d[:])
        waT = const.tile([FD, P], F32R)
        tp = ps_tr.tile([FD, P], F32, tag="tr")
        nc.tensor.transpose(tp[:], wa_nat[:], ident[:P, :P])
        nc.scalar.copy(waT[:], tp[:])

        # hidden row 0 per batch, in column layout [128, HC] (h = c*128+p)
        hrow0 = const.tile([128, HC, BPC], F32)
        for b in range(BPC):
            src = hid_d[b, 0, :].rearrange("(c p) -> p c", p=128)
            nc.sync.dma_start(hrow0[:, :, b], src)

        # ln/db replicated across BPC partitions
        db1 = const.tile([1, H], F32)
        lnw1 = const.tile([1, H], F32)
        lnb1 = const.tile([1, H], F32)
        nc.sync.dma_start(db1[:], db_d[:].rearrange("(o h) -> o h", o=1))
        nc.sync.dma_start(lnw1[:], lnw_d[:].rearrange("(o h) -> o h", o=1))
        nc.sync.dma_start(lnb1[:], lnb_d[:].rearrange("(o h) -> o h", o=1))
        db_r = const.tile([BPC, H], F32)
        lnw_r = const.tile([BPC, H], F32)
        lnb_r = const.tile([BPC, H], F32)
        nc.gpsimd.partition_broadcast(db_r[:], db1[:], channels=BPC)
        nc.gpsimd.partition_broadcast(lnw_r[:], lnw1[:], channels=BPC)
        nc.gpsimd.partition_broadcast(lnb_r[:], lnb1[:], channels=BPC)

        # ---------------- persistent state ----------------
        hid_all = persist.tile([128, BPC, SC, H], F32)      # natural layout
        textT_all = persist.tile([P, BPC, S], F32R)          # raw text proj
        ssq_all = persist.tile([P, BPC], F32)
        fd_all = persist.tile([128, HC, BPC], F32R)

        # ---------------- phase 1: load hidden, transpose, text proj ----
        for b in range(BPC):
            for i in range(SC):
                nc.sync.dma_start(hid_all[:, b, i, :],
                                  hid_d[b, i * 128:(i + 1) * 128, :])
            tx_ps = ps_mm.tile([P, S], F32, tag="proj")
            for c in range(HC):
                hT = work.tile([128, S], F32R, tag="hidT", bufs=2)
                trp = ps_tr.tile([128, S], F32, tag="tr")
                for i in range(SC):
                    nc.tensor.transpose(
                        trp[:, i * 128:(i + 1) * 128],
                        hid_all[:, b, i, c * 128:(c + 1) * 128],
                        ident[:, :])
                if c % 2 == 0:
                    nc.scalar.copy(hT[:], trp[:])
                else:
                    nc.vector.tensor_copy(hT[:], trp[:])
                nc.tensor.matmul(tx_ps[:], wtT[:, c, :], hT[:],
                                 start=(c == 0), stop=(c == HC - 1))
            nc.scalar.copy(textT_all[:, b, :], tx_ps[:])
            sq_ps = ps_mm.tile([P, S], F32, tag="proj")
            nc.scalar.activation(sq_ps[:], tx_ps[:], AF.Square,
                                 accum_out=ssq_all[:, b:b + 1])

        # ---------------- global sum-of-squares (AllReduce) ----------------
        ssq_vec = smalls.tile([P, 1], F32, bufs=1)
        nc.vector.reduce_sum(ssq_vec[:], ssq_all[:], axis=mybir.AxisListType.X)
        ssq_ps = ps_mm.tile([1, 1], F32, tag="proj")
        nc.tensor.matmul(ssq_ps[:], ones_p[:], ssq_vec[:])
        ssq_sc = smalls.tile([1, 1], F32, bufs=1)
        nc.scalar.copy(ssq_sc[:], ssq_ps[:])
        cc_in = dram.tile([1, 1], F32)
        cc_out = dram.tile([1, 1], F32)
        nc.gpsimd.dma_start(cc_in[:], ssq_sc[:])
        nc.gpsimd.collective_compute(
            "AllReduce", ALU.add,
            replica_groups=[list(range(N_CORES))],
            ins=[cc_in.opt()], outs=[cc_out.opt()])
        ssq_g = smalls.tile([1, 1], F32, bufs=1)
        nc.gpsimd.dma_start(ssq_g[:], cc_out[:])
        w2 = smalls.tile([1, 1], F32, bufs=1)
        nc.scalar.sqrt(w2[:], ssq_g[:])
        inv_w2 = smalls.tile([1, 1], F32, bufs=1)
        nc.vector.reciprocal(inv_w2[:], w2[:])
        inv_w2_r = smalls.tile([128, 1], F32, bufs=1)
        nc.gpsimd.partition_broadcast(inv_w2_r[:], inv_w2[:])

        # ---------------- dW^T build (fills the collective window) --------
        dwT = persist.tile([128, HC, H], F32R)
        for oc in range(HC):
            dw_nat = work.tile([128, H], F32, tag="dwnat", bufs=1)
            nc.sync.dma_start(dw_nat[:], dw_d[oc * 128:(oc + 1) * 128, :])
            for hc in range(HC):
                tp2 = ps_tr.tile([128, 128], F32, tag="tr")
                nc.tensor.transpose(tp2[:], dw_nat[:, hc * 128:(hc + 1) * 128],
                                    ident[:, :])
                if (oc + hc) % 2 == 0:
                    nc.scalar.copy(dwT[:, hc, oc * 128:(oc + 1) * 128], tp2[:])
                else:
                    nc.vector.tensor_copy(dwT[:, hc, oc * 128:(oc + 1) * 128],
                                          tp2[:])

        # ---------------- per-batch: audio proj, grams, softmax, matvec ----
        for b in range(BPC):
            # audio: load + transpose + project -> audT [P, S]
            aud_ps = ps_tr.tile([FD, S], F32, tag="tr")
            for i in range(SC):
                ach = work.tile([128, FD], F32, tag="ach", bufs=2)
                nc.sync.dma_start(ach[:], aud_d[b, i * 128:(i + 1) * 128, :])
                nc.tensor.transpose(aud_ps[:, i * 128:(i + 1) * 128],
                                    ach[:], ident[:, :])
            audT_raw = work.tile([FD, S], F32R, tag="audraw", bufs=2)
            if b % 2 == 0:
                nc.scalar.copy(audT_raw[:], aud_ps[:])
            else:
                nc.vector.tensor_copy(audT_raw[:], aud_ps[:])
            ap_ps = ps_mm.tile([P, S], F32, tag="proj")
            nc.tensor.matmul(ap_ps[:], waT[:], audT_raw[:])
            audT = work.tile([P, S], F32R, tag="audT", bufs=3)
            nc.scalar.copy(audT[:], ap_ps[:])

            am_b = smalls.tile([1, S], F32, tag="amb", bufs=2)
            nc.sync.dma_start(am_b[:], am_d[b:b + 1, :])

            for m in range(SC):
                msl = slice(m * 128, (m + 1) * 128)
                gt_ps = ps_g.tile([128, S], F32, tag="g")
                nc.tensor.matmul(gt_ps[:], textT_all[:, b, msl],
                                 textT_all[:, b, :])
                a_t = att.tile([128, S], F32, tag="att")
                nc.scalar.activation(a_t[:], gt_ps[:], AF.Relu,
                                     scale=inv_w2_r[:])
                nc.sync.dma_start(tatt_d[b, msl, :], a_t[:])

                ga_ps = ps_g.tile([128, S], F32, tag="g")
                nc.tensor.matmul(ga_ps[:], audT[:, msl], audT[:])
                r_t = att.tile([128, S], F32, tag="att")
                if aw >= 0.0:
                    if aw == 1.0 and (b * SC + m) % 2 == 1:
                        nc.vector.tensor_scalar_max(r_t[:], ga_ps[:], 0.0)
                    else:
                        nc.scalar.activation(r_t[:], ga_ps[:], AF.Relu,
                                             scale=float(aw))
                else:
                    nc.scalar.activation(r_t[:], ga_ps[:], AF.Relu)
                    r2 = att.tile([128, S], F32, tag="att")
                    nc.vector.tensor_scalar_mul(r2[:], r_t[:], float(aw))
                    r_t = r2

                f_pre = att.tile([128, S], F32, tag="att")
                if tw == 1.0:
                    nc.vector.tensor_add(f_pre[:], a_t[:], r_t[:])
                else:
                    nc.vector.scalar_tensor_tensor(
                        f_pre[:], a_t[:], float(tw), r_t[:],
                        op0=ALU.mult, op1=ALU.add)

                if fb == 0.0 and tw >= 0.0 and aw >= 0.0:
                    f_out = f_pre
                elif tw >= 0.0 and aw >= 0.0 and fb >= 0.0:
                    f_out = att.tile([128, S], F32, tag="att")
                    nc.scalar.add(f_out[:], f_pre[:], float(fb))
                else:
                    f_out = att.tile([128, S], F32, tag="att")
                    fb_t = smalls.tile([128, 1], F32, tag="fbt", bufs=1)
                    nc.gpsimd.memset(fb_t[:], float(fb))
                    nc.scalar.activation(f_out[:], f_pre[:], AF.Relu,
                                         bias=fb_t[:])
                nc.sync.dma_start(fatt_d[b, msl, :], f_out[:])

                if m == 0:
                    z = smalls.tile([1, S], F32, tag="z", bufs=1)
                    nc.vector.tensor_add(z[:], f_pre[0:1, :], am_b[:])
                    mx = smalls.tile([1, 1], F32, tag="mx")
                    nc.vector.reduce_max(mx[:], z[:], axis=mybir.AxisListType.X)
                    mneg = smalls.tile([1, 1], F32, tag="mneg")
                    nc.vector.tensor_scalar_mul(mneg[:], mx[:], -1.0)
                    e_row = smalls.tile([1, S], F32, tag="erow", bufs=1)
                    sumexp = smalls.tile([1, 1], F32, tag="sumexp")
                    nc.scalar.activation(e_row[:], z[:], AF.Exp,
                                         bias=mneg[:], accum_out=sumexp[:])
                    rcp = smalls.tile([1, 1], F32, tag="rcp")
                    nc.vector.reciprocal(rcp[:], sumexp[:])
                    rcp_r = smalls.tile([128, 1], F32, tag="rcpr")
                    nc.gpsimd.partition_broadcast(rcp_r[:], rcp[:])

                    ecol_ps = ps_mm.tile([128, SC], F32, tag="proj")
                    for i in range(SC):
                        nc.tensor.transpose(
                            ecol_ps[:, i:i + 1],
                            e_row[0:1, i * 128:(i + 1) * 128],
                            ident[:1, :1])
                    ecol = smalls.tile([128, SC], F32, tag="ecol")
                    nc.vector.tensor_copy(ecol[:], ecol_ps[:])

                    fd_ps = ps_mm.tile([128, HC], F32, tag="proj")
                    for c in range(HC):
                        for i in range(SC):
                            nc.tensor.matmul(
                                fd_ps[:, c:c + 1],
                                hid_all[:, b, i, c * 128:(c + 1) * 128],
                                ecol[:, i:i + 1],
                                start=(i == 0), stop=(i == SC - 1))
                    nc.vector.scalar_tensor_tensor(
                        fd_all[:, :, b], fd_ps[:], rcp_r[:, 0:1],
                        hrow0[:, :, b],
                        op0=ALU.mult, op1=ALU.add)

        # ---------------- dense + layernorm on row-0 states ----------------
        h0_ps = ps_x.tile([BPC, H], F32, tag="h0")
        for c in range(HC):
            nc.tensor.matmul(h0_ps[:, 0:512], fd_all[:, c, :],
                             dwT[:, c, 0:512],
                             start=(c == 0), stop=(c == HC - 1))
            nc.tensor.matmul(h0_ps[:, 512:H], fd_all[:, c, :],
                             dwT[:, c, 512:H],
                             start=(c == 0), stop=(c == HC - 1))
        xb = smalls.tile([BPC, H], F32, tag="xln", bufs=2)
        nc.vector.tensor_add(xb[:], h0_ps[:], db_r[:])
        usum = smalls.tile([BPC, 1], F32, tag="usum")
        nc.vector.reduce_sum(usum[:], xb[:], axis=mybir.AxisListType.X)
        uneg = smalls.tile([BPC, 1], F32, tag="uneg")
        nc.vector.tensor_scalar_mul(uneg[:], usum[:], -1.0 / H)
        xc = smalls.tile([BPC, H], F32, tag="xln", bufs=2)
        nc.vector.tensor_scalar_add(xc[:], xb[:], uneg[:])
        sq2 = smalls.tile([BPC, H], F32, tag="xln", bufs=2)
        v = smalls.tile([BPC, 1], F32, tag="v")
        nc.scalar.activation(sq2[:], xc[:], AF.Square, accum_out=v[:])
        eps_t = smalls.tile([BPC, 1], F32, tag="eps", bufs=1)
        nc.gpsimd.memset(eps_t[:], float(LN_EPS))
        std = smalls.tile([BPC, 1], F32, tag="std")
        nc.scalar.activation(std[:], v[:], AF.Sqrt, scale=1.0 / H,
                             bias=eps_t[:])
        rstd = smalls.tile([BPC, 1], F32, tag="rstd")
        nc.vector.reciprocal(rstd[:], std[:])
        y1 = smalls.tile([BPC, H], F32, tag="xln", bufs=2)
        nc.vector.scalar_tensor_tensor(y1[:], xc[:], rstd[:, 0:1], lnw_r[:],
                                       op0=ALU.mult, op1=ALU.mult)
        y2 = smalls.tile([BPC, H], F32, tag="xln", bufs=2)
        nc.vector.tensor_add(y2[:], y1[:], lnb_r[:])
        nc.sync.dma_start(out0_d[:], y2[:])


# ------------------------------------------------------------------
# NTFF profiling hook (only used when PROFILE=True)
# ------------------------------------------------------------------
def _install_profile_hook():
    try:
        import antenv.axon_hooks  # noqa
        return
    except ImportError:
        pass
    so_path = "/opt/axon/libaxon_pjrt.so"
    try:
        lib = ctypes.CDLL(so_path)
    except OSError:
        return
    if not hasattr(lib, "axon_start_nrt_profile"):
        return
    lib.axon_start_nrt_profile.argtypes = [ctypes.POINTER(ctypes.c_int64),
                                           ctypes.c_size_t]
    lib.axon_start_nrt_profile.restype = ctypes.c_int64
    lib.axon_stop_nrt_profile.argtypes = [ctypes.c_char_p]
    lib.axon_stop_nrt_profile.restype = ctypes.c_int64

    @contextlib.contextmanager
    def _hook(output_dir, device_ids):
        import jax
        jax.devices()
        if device_ids:
            ids = (ctypes.c_int64 * len(device_ids))(*device_ids)
            rc = lib.axon_start_nrt_profile(ids, len(device_ids))
        else:
            rc = lib.axon_start_nrt_profile(None, 0)
        if rc != 0:
            raise RuntimeError(f"axon_start_nrt_profile rc={rc}")
        try:
            yield
        finally:
            n = lib.axon_stop_nrt_profile(str(output_dir).encode())
            if n < 0:
                raise RuntimeError(f"axon_stop_nrt_profile rc={n}")

    mod = types.ModuleType("antenv.axon_hooks")
    _hook_box = [_hook]
    mod.get_axon_ntff_profile_hook = lambda: _hook_box[0]
    mod.set_axon_ntff_profile_hook = lambda h: _hook_box.__setitem__(0, h)
    sys.modules["antenv.axon_hooks"] = mod
    import antenv
    antenv.axon_hooks = mod


# ------------------------------------------------------------------
# Host wrapper
# ------------------------------------------------------------------
def kernel(hidden_states, audio_data, attention_mask, Wt, Wa, text_w, audio_w,
           fbias, dense_W, dense_b, ln_w, ln_b):
    global LAST_EXEC_NS
    hs = np.ascontiguousarray(np.asarray(hidden_states, np.float32))
    ad = np.ascontiguousarray(np.asarray(audio_data, np.float32))
    am = np.ascontiguousarray(
        np.asarray(attention_mask, np.float32).reshape(B, S))
    wt = np.ascontiguousarray(np.asarray(Wt, np.float32))
    wa = np.ascontiguousarray(np.asarray(Wa, np.float32))
    dw = np.ascontiguousarray(np.asarray(dense_W, np.float32))
    db = np.ascontiguousarray(np.asarray(dense_b, np.float32))
    lnw = np.ascontiguousarray(np.asarray(ln_w, np.float32))
    lnb = np.ascontiguousarray(np.asarray(ln_b, np.float32))
    tw = float(np.asarray(text_w).reshape(-1)[0])
    aw = float(np.asarray(audio_w).reshape(-1)[0])
    fb = float(np.asarray(fbias).reshape(-1)[0])

    key = (tw, aw, fb)
    if key not in _compiled:
        _compiled[key] = _build(tw, aw, fb)
    nc = _compiled[key]

    in_maps = []
    for i in range(N_CORES):
        sl = slice(i * BPC, (i + 1) * BPC)
        in_maps.append({
            "hid": hs[sl], "aud": ad[sl], "am": am[sl],
            "wt": wt, "wa": wa, "dw": dw, "db": db,
            "lnw": lnw, "lnb": lnb,
        })

    if PROFILE:
        _install_profile_hook()
    res = run_bass_kernel_spmd(nc, in_maps, list(range(N_CORES)),
                               trace=PROFILE)
    LAST_EXEC_NS = res.exec_time_ns

    h0 = np.concatenate([res.results[i]["out0"] for i in range(N_CORES)], 0)
    t_att = np.concatenate([res.results[i]["t_att"] for i in range(N_CORES)], 0)
    f_att = np.concatenate([res.results[i]["f_att"] for i in range(N_CORES)], 0)
    return h0, t_att, f_att


# revision 10
# speedup vs baseline: 1.2805x; 1.2805x over previous
"""Trainium2 Bass kernel for the multimodal BERT fusion block.

Contract: kernel(**inputs) takes FULL unsharded numpy inputs (as produced by
setup_inputs()), runs an SPMD Bass kernel on 8 NeuronCores (data-parallel over
the batch dim, params replicated), and returns the FULL outputs
(h[:,0], text_att1, fusion_att1) as numpy arrays.

Math per batch b (S=512 tokens, H=768, P=30 proj dim, FD=74 audio feat):
  textT[b]  = Wt @ hidden[b]^T                    [30, 512]
  ssq       = sum(textT^2) over ALL batches       (global -> AllReduce)
  inv_w2    = ssq^-0.5   (w = ssq^0.25; text/w gram scale = 1/w^2)
  text_att1 = relu((textT^T textT) * inv_w2)      [512, 512]   (output)
  audT[b]   = Wa @ audio[b]^T                     [30, 512]
  audio_att = relu(audT^T audT)
  F_pre     = tw*text_att1 + aw*audio_att ; fusion_att1 = relu(F_pre+fb) (output)
  row0: softmax(F_pre[0,:] + am[b,:]) -> e/sum ;  fd0 = e@hidden[b]/sum + hidden[b,0]
  h0 = LN(fd0 @ dW^T + db) * ln_w + ln_b          [768]        (output h[:,0])

Implementation notes:
  - hidden^T is built on-chip via PE transposes (f32 DMA transpose does not
    exist on trn2) and kept resident in fp32r; it feeds both the text
    projection (PE, fp32r) and the softmax-weighted row-0 reduction (DVE
    tensor_tensor_reduce dot products).
  - All large matmuls run in fp32r (1 cycle/row at N>=256 vs 4 for fp32).
  - A tiny scalar AllReduce distributes the global text norm.
"""
import ctypes
import contextlib
import sys
import types

import numpy as np

import concourse.bass as bass
import concourse.bacc as bacc
import concourse.mybir as mybir
import concourse.tile as tile
from concourse import masks
from concourse.bass_utils import run_bass_kernel_spmd

B, S, H, FD, P = 64, 512, 768, 74, 30
N_CORES = 8
BPC = B // N_CORES          # 8 batches per core
HC, SC = H // 128, S // 128  # 6, 4
LN_EPS = 1e-12

F32 = mybir.dt.float32
F32R = mybir.dt.float32r
AF = mybir.ActivationFunctionType
ALU = mybir.AluOpType
AX = mybir.AxisListType

PROFILE = False          # set True (from test.py) to capture HW exec time
LAST_EXEC_NS = None

_compiled = {}


def _build(tw: float, aw: float, fb: float):
    nc = bacc.Bacc("TRN2", target_bir_lowering=False, debug=False,
                   num_devices=N_CORES)

    hid_d = nc.dram_tensor("hid", [BPC, S, H], F32, kind="ExternalInput").ap()
    aud_d = nc.dram_tensor("aud", [BPC, S, FD], F32, kind="ExternalInput").ap()
    am_d = nc.dram_tensor("am", [BPC, S], F32, kind="ExternalInput").ap()
    wt_d = nc.dram_tensor("wt", [P, H], F32, kind="ExternalInput").ap()
    wa_d = nc.dram_tensor("wa", [P, FD], F32, kind="ExternalInput").ap()
    dw_d = nc.dram_tensor("dw", [H, H], F32, kind="ExternalInput").ap()
    db_d = nc.dram_tensor("db", [H], F32, kind="ExternalInput").ap()
    lnw_d = nc.dram_tensor("lnw", [H], F32, kind="ExternalInput").ap()
    lnb_d = nc.dram_tensor("lnb", [H], F32, kind="ExternalInput").ap()

    tatt_d = nc.dram_tensor("t_att", [BPC, S, S], F32, kind="ExternalOutput").ap()
    fatt_d = nc.dram_tensor("f_att", [BPC, S, S], F32, kind="ExternalOutput").ap()
    out0_d = nc.dram_tensor("out0", [BPC, H], F32, kind="ExternalOutput").ap()

    with tile.TileContext(nc) as tc:
        _body(nc, tc, tw, aw, fb,
              hid_d, aud_d, am_d, wt_d, wa_d, dw_d, db_d, lnw_d, lnb_d,
              tatt_d, fatt_d, out0_d)
    nc.compile()
    return nc


def _body(nc, tc, tw, aw, fb,
          hid_d, aud_d, am_d, wt_d, wa_d, dw_d, db_d, lnw_d, lnb_d,
          tatt_d, fatt_d, out0_d):
    from contextlib import ExitStack
    ctx = ExitStack()
    with ctx:
        const = ctx.enter_context(tc.tile_pool(name="const", bufs=1))
        persist = ctx.enter_context(tc.tile_pool(name="persist", bufs=1))
        work = ctx.enter_context(tc.tile_pool(name="work", bufs=2))
        att = ctx.enter_context(tc.tile_pool(name="att", bufs=4))
        smalls = ctx.enter_context(tc.tile_pool(name="smalls", bufs=2))
        dram = ctx.enter_context(tc.tile_pool(name="dram", bufs=1, space="DRAM"))

        ps_tr = ctx.enter_context(tc.tile_pool(name="ps_tr", bufs=3, space="PSUM"))
        ps_mm = ctx.enter_context(tc.tile_pool(name="ps_mm", bufs=2, space="PSUM"))
        ps_g = ctx.enter_context(tc.tile_pool(name="ps_g", bufs=2, space="PSUM"))

        # ---------------- constants / params ----------------
        ident = const.tile([128, 128], F32)
        masks.make_identity(nc, ident[:])

        ones_p = const.tile([P, 1], F32)
        nc.gpsimd.memset(ones_p[:], 1.0)

        # Wt^T: [P, H] -> HC chunks [128, P]
        wt_nat = const.tile([P, H], F32)
        nc.sync.dma_start(wt_nat[:], wt_d[:])
        wtT = const.tile([128, HC, P], F32R)
        for c in range(HC):
            tp = ps_tr.tile([128, P], F32, tag="tr")
            nc.tensor.transpose(tp[:], wt_nat[:, c * 128:(c + 1) * 128],
                                ident[:P, :P])
            nc.scalar.copy(wtT[:, c, :], tp[:])

        # Wa^T: [P, FD] -> [FD, P]
        wa_nat = const.tile([P, FD], F32)
        nc.sync.dma_start(wa_nat[:], wa_d[:])
        waT = const.tile([FD, P], F32R)
        tpw = ps_tr.tile([FD, P], F32, tag="tr")
        nc.tensor.transpose(tpw[:], wa_nat[:], ident[:P, :P])
        nc.scalar.copy(waT[:], tpw[:])

        # ln/db replicated across BPC partitions via broadcast-read DMA
        db_r = const.tile([BPC, H], F32)
        lnw_r = const.tile([BPC, H], F32)
        lnb_r = const.tile([BPC, H], F32)
        for dst, src in ((db_r, db_d), (lnw_r, lnw_d), (lnb_r, lnb_d)):
            s1 = src[:].rearrange("(o h) -> o h", o=1)
            nc.sync.dma_start(dst[:], s1.to_broadcast((BPC, H)))

        # ---------------- persistent state ----------------
        hidT_all = persist.tile([128, BPC, HC, S], F32R)    # hidden^T, fp32r
        textT_all = persist.tile([P, BPC, S], F32R)         # raw text proj
        ssq_all = persist.tile([P, BPC], F32)
        fd_all = persist.tile([128, HC, BPC], F32R)

        # ---------------- phase 1: load hidden, transpose, text proj ----
        for b in range(BPC):
            hid_nat = work.tile([128, SC, H], F32, tag="hidnat", bufs=2)
            for i in range(SC):
                nc.sync.dma_start(hid_nat[:, i, :],
                                  hid_d[b, i * 128:(i + 1) * 128, :])
            tx_ps = ps_mm.tile([P, S], F32, tag="proj")
            for c in range(HC):
                for i in range(SC):
                    trp = ps_tr.tile([128, 128], F32, tag="tr")
                    nc.tensor.transpose(
                        trp[:], hid_nat[:, i, c * 128:(c + 1) * 128],
                        ident[:, :])
                    dst = hidT_all[:, b, c, i * 128:(i + 1) * 128]
                    if (c + i) % 2 == 0:
                        nc.scalar.copy(dst, trp[:])
                    else:
                        nc.vector.tensor_copy(dst, trp[:])
                nc.tensor.matmul(tx_ps[:], wtT[:, c, :], hidT_all[:, b, c, :],
                                 start=(c == 0), stop=(c == HC - 1))
            nc.scalar.copy(textT_all[:, b, :], tx_ps[:])
            sq_ps = ps_mm.tile([P, S], F32, tag="proj")
            nc.scalar.activation(sq_ps[:], tx_ps[:], AF.Square,
                                 accum_out=ssq_all[:, b:b + 1])

        # ---------------- global sum-of-squares (AllReduce) ----------------
        ssq_vec = smalls.tile([P, 1], F32, bufs=1)
        nc.vector.reduce_sum(ssq_vec[:], ssq_all[:], axis=AX.X)
        ssq_ps = ps_mm.tile([1, 1], F32, tag="proj")
        nc.tensor.matmul(ssq_ps[:], ones_p[:], ssq_vec[:])
        ssq_sc = smalls.tile([1, 1], F32, bufs=1)
        nc.scalar.copy(ssq_sc[:], ssq_ps[:])
        cc_in = dram.tile([1, 1], F32)
        cc_out = dram.tile([1, 1], F32)
        nc.gpsimd.dma_start(cc_in[:], ssq_sc[:])
        nc.gpsimd.collective_compute(
            "AllReduce", ALU.add,
            replica_groups=[list(range(N_CORES))],
            ins=[cc_in.opt()], outs=[cc_out.opt()])
        ssq_g = smalls.tile([1, 1], F32, bufs=1)
        nc.gpsimd.dma_start(ssq_g[:], cc_out[:])
        w2 = smalls.tile([1, 1], F32, bufs=1)
        nc.scalar.sqrt(w2[:], ssq_g[:])
        inv_w2 = smalls.tile([1, 1], F32, bufs=1)
        nc.vector.reciprocal(inv_w2[:], w2[:])
        inv_w2_r = smalls.tile([128, 1], F32, bufs=1)
        nc.gpsimd.partition_broadcast(inv_w2_r[:], inv_w2[:])

        # ---------------- dW^T build (fills the collective window) --------
        dwT = persist.tile([128, HC, H], F32R)
        for oc in range(HC):
            dw_nat = work.tile([128, H], F32, tag="dwnat", bufs=1)
            nc.sync.dma_start(dw_nat[:], dw_d[oc * 128:(oc + 1) * 128, :])
            for hc in range(HC):
                tp2 = ps_tr.tile([128, 128], F32, tag="tr")
                nc.tensor.transpose(tp2[:], dw_nat[:, hc * 128:(hc + 1) * 128],
                                    ident[:, :])
                if (oc + hc) % 2 == 0:
                    nc.scalar.copy(dwT[:, hc, oc * 128:(oc + 1) * 128], tp2[:])
                else:
                    nc.vector.tensor_copy(dwT[:, hc, oc * 128:(oc + 1) * 128],
                                          tp2[:])

        # ---------------- per-batch: audio proj, grams, softmax, dots ----
        for b in range(BPC):
            # audio: load + transpose + project -> audT [P, S]
            aud_ps = ps_tr.tile([FD, S], F32, tag="atr", bufs=1)
            for i in range(SC):
                ach = work.tile([128, FD], F32, tag="ach", bufs=2)
                nc.sync.dma_start(ach[:], aud_d[b, i * 128:(i + 1) * 128, :])
                nc.tensor.transpose(aud_ps[:, i * 128:(i + 1) * 128],
                                    ach[:], ident[:, :])
            audT_raw = work.tile([FD, S], F32R, tag="audraw", bufs=2)
            if b % 2 == 0:
                nc.scalar.copy(audT_raw[:], aud_ps[:])
            else:
                nc.vector.tensor_copy(audT_raw[:], aud_ps[:])
            ap_ps = ps_mm.tile([P, S], F32, tag="proj")
            nc.tensor.matmul(ap_ps[:], waT[:], audT_raw[:])
            audT = work.tile([P, S], F32R, tag="audT", bufs=3)
            nc.scalar.copy(audT[:], ap_ps[:])

            am_b = smalls.tile([1, S], F32, tag="amb", bufs=1)
            nc.sync.dma_start(am_b[:], am_d[b:b + 1, :])

            for m in range(SC):
                msl = slice(m * 128, (m + 1) * 128)
                gt_ps = ps_g.tile([128, S], F32, tag="g")
                nc.tensor.matmul(gt_ps[:], textT_all[:, b, msl],
                                 textT_all[:, b, :])
                a_t = att.tile([128, S], F32, tag="att")
                nc.scalar.activation(a_t[:], gt_ps[:], AF.Relu,
                                     scale=inv_w2_r[:])
                nc.sync.dma_start(tatt_d[b, msl, :], a_t[:])

                ga_ps = ps_g.tile([128, S], F32, tag="g")
                nc.tensor.matmul(ga_ps[:], audT[:, msl], audT[:])
                r_t = att.tile([128, S], F32, tag="att")
                if aw >= 0.0:
                    if aw == 1.0 and (b * SC + m) % 2 == 1:
                        nc.vector.tensor_scalar_max(r_t[:], ga_ps[:], 0.0)
                    else:
                        nc.scalar.activation(r_t[:], ga_ps[:], AF.Relu,
                                             scale=float(aw))
                else:
                    nc.scalar.activation(r_t[:], ga_ps[:], AF.Relu)
                    r2 = att.tile([128, S], F32, tag="att")
                    nc.vector.tensor_scalar_mul(r2[:], r_t[:], float(aw))
                    r_t = r2

                f_pre = att.tile([128, S], F32, tag="att")
                if tw == 1.0:
                    nc.vector.tensor_add(f_pre[:], a_t[:], r_t[:])
                else:
                    nc.vector.scalar_tensor_tensor(
                        f_pre[:], a_t[:], float(tw), r_t[:],
                        op0=ALU.mult, op1=ALU.add)

                if fb == 0.0 and tw >= 0.0 and aw >= 0.0:
                    f_out = f_pre
                elif tw >= 0.0 and aw >= 0.0 and fb >= 0.0:
                    f_out = att.tile([128, S], F32, tag="att")
                    nc.scalar.add(f_out[:], f_pre[:], float(fb))
                else:
                    f_out = att.tile([128, S], F32, tag="att")
                    fb_t = smalls.tile([128, 1], F32, tag="fbt", bufs=1)
                    nc.gpsimd.memset(fb_t[:], float(fb))
                    nc.scalar.activation(f_out[:], f_pre[:], AF.Relu,
                                         bias=fb_t[:])
                nc.sync.dma_start(fatt_d[b, msl, :], f_out[:])

                if m == 0:
                    z = smalls.tile([1, S], F32, tag="z", bufs=1)
                    nc.vector.tensor_add(z[:], f_pre[0:1, :], am_b[:])
                    mx = smalls.tile([1, 1], F32, tag="mx")
                    nc.vector.reduce_max(mx[:], z[:], axis=AX.X)
                    mneg = smalls.tile([1, 1], F32, tag="mneg")
                    nc.vector.tensor_scalar_mul(mneg[:], mx[:], -1.0)
                    e_row = smalls.tile([1, S], F32, tag="erow", bufs=1)
                    sumexp = smalls.tile([1, 1], F32, tag="sumexp")
                    nc.scalar.activation(e_row[:], z[:], AF.Exp,
                                         bias=mneg[:], accum_out=sumexp[:])
                    rcp = smalls.tile([1, 1], F32, tag="rcp")
                    nc.vector.reciprocal(rcp[:], sumexp[:])
                    rcp_r = smalls.tile([128, 1], F32, tag="rcpr")
                    nc.gpsimd.partition_broadcast(rcp_r[:], rcp[:])
                    e_rep = work.tile([128, S], F32, tag="erep", bufs=1)
                    nc.gpsimd.partition_broadcast(e_rep[:], e_row[:])

                    # fd0 numerator via DVE dot products against hidden^T
                    fdnum = smalls.tile([128, HC], F32, tag="fdnum")
                    for c in range(HC):
                        scr = work.tile([128, S], F32, tag="ttrscr", bufs=1)
                        nc.vector.scalar_tensor_tensor(
                            scr[:], hidT_all[:, b, c, :].bitcast(F32),
                            1.0, e_rep[:],
                            op0=ALU.mult, op1=ALU.mult,
                            accum_out=fdnum[:, c:c + 1])
                    nc.vector.scalar_tensor_tensor(
                        fd_all[:, :, b], fdnum[:], rcp_r[:, 0:1],
                        hidT_all[:, b, :, 0].bitcast(F32),
                        op0=ALU.mult, op1=ALU.add)

        # ---------------- dense + layernorm on row-0 states ----------------
        h0a = ps_g.tile([BPC, 512], F32, tag="g")
        h0b = ps_g.tile([BPC, H - 512], F32, tag="g")
        for c in range(HC):
            nc.tensor.matmul(h0a[:], fd_all[:, c, :], dwT[:, c, 0:512],
                             start=(c == 0), stop=(c == HC - 1))
            nc.tensor.matmul(h0b[:], fd_all[:, c, :], dwT[:, c, 512:H],
                             start=(c == 0), stop=(c == HC - 1))
        xb = smalls.tile([BPC, H], F32, tag="xln", bufs=2)
        nc.vector.tensor_add(xb[:, 0:512], h0a[:], db_r[:, 0:512])
        nc.vector.tensor_add(xb[:, 512:H], h0b[:], db_r[:, 512:H])
        usum = smalls.tile([BPC, 1], F32, tag="usum")
        nc.vector.reduce_sum(usum[:], xb[:], axis=AX.X)
        uneg = smalls.tile([BPC, 1], F32, tag="uneg")
        nc.vector.tensor_scalar_mul(uneg[:], usum[:], -1.0 / H)
        xc = smalls.tile([BPC, H], F32, tag="xln", bufs=2)
        nc.vector.tensor_scalar_add(xc[:], xb[:], uneg[:])
        sq2 = smalls.tile([BPC, H], F32, tag="xln", bufs=2)
        v = smalls.tile([BPC, 1], F32, tag="v")
        nc.scalar.activation(sq2[:], xc[:], AF.Square, accum_out=v[:])
        eps_t = smalls.tile([BPC, 1], F32, tag="eps", bufs=1)
        nc.gpsimd.memset(eps_t[:], float(LN_EPS))
        std = smalls.tile([BPC, 1], F32, tag="std")
        nc.scalar.activation(std[:], v[:], AF.Sqrt, scale=1.0 / H,
                             bias=eps_t[:])
        rstd = smalls.tile([BPC, 1], F32, tag="rstd")
        nc.vector.reciprocal(rstd[:], std[:])
        y1 = smalls.tile([BPC, H], F32, tag="xln", bufs=2)
        nc.vector.scalar_tensor_tensor(y1[:], xc[:], rstd[:, 0:1], lnw_r[:],
                                       op0=ALU.mult, op1=ALU.mult)
        y2 = smalls.tile([BPC, H], F32, tag="xln", bufs=2)
        nc.vector.tensor_add(y2[:], y1[:], lnb_r[:])
        nc.sync.dma_start(out0_d[:], y2[:])


# ------------------------------------------------------------------
# NTFF profiling hook (only used when PROFILE=True)
# ------------------------------------------------------------------
def _install_profile_hook():
    try:
        import antenv.axon_hooks  # noqa
        return
    except ImportError:
        pass
    so_path = "/opt/axon/libaxon_pjrt.so"
    try:
        lib = ctypes.CDLL(so_path)
    except OSError:
        return
    if not hasattr(lib, "axon_start_nrt_profile"):
        return
    lib.axon_start_nrt_profile.argtypes = [ctypes.POINTER(ctypes.c_int64),
                                           ctypes.c_size_t]
    lib.axon_start_nrt_profile.restype = ctypes.c_int64
    lib.axon_stop_nrt_profile.argtypes = [ctypes.c_char_p]
    lib.axon_stop_nrt_profile.restype = ctypes.c_int64

    @contextlib.contextmanager
    def _hook(output_dir, device_ids):
        import jax
        jax.devices()
        if device_ids:
            ids = (ctypes.c_int64 * len(device_ids))(*device_ids)
            rc = lib.axon_start_nrt_profile(ids, len(device_ids))
        else:
            rc = lib.axon_start_nrt_profile(None, 0)
        if rc != 0:
            raise RuntimeError(f"axon_start_nrt_profile rc={rc}")
        try:
            yield
        finally:
            n = lib.axon_stop_nrt_profile(str(output_dir).encode())
            if n < 0:
                raise RuntimeError(f"axon_stop_nrt_profile rc={n}")

    mod = types.ModuleType("antenv.axon_hooks")
    _hook_box = [_hook]
    mod.get_axon_ntff_profile_hook = lambda: _hook_box[0]
    mod.set_axon_ntff_profile_hook = lambda h: _hook_box.__setitem__(0, h)
    sys.modules["antenv.axon_hooks"] = mod
    import antenv
    antenv.axon_hooks = mod


# ------------------------------------------------------------------
# Host wrapper
# ------------------------------------------------------------------
def kernel(hidden_states, audio_data, attention_mask, Wt, Wa, text_w, audio_w,
           fbias, dense_W, dense_b, ln_w, ln_b):
    global LAST_EXEC_NS
    hs = np.ascontiguousarray(np.asarray(hidden_states, np.float32))
    ad = np.ascontiguousarray(np.asarray(audio_data, np.float32))
    am = np.ascontiguousarray(
        np.asarray(attention_mask, np.float32).reshape(B, S))
    wt = np.ascontiguousarray(np.asarray(Wt, np.float32))
    wa = np.ascontiguousarray(np.asarray(Wa, np.float32))
    dw = np.ascontiguousarray(np.asarray(dense_W, np.float32))
    db = np.ascontiguousarray(np.asarray(dense_b, np.float32))
    lnw = np.ascontiguousarray(np.asarray(ln_w, np.float32))
    lnb = np.ascontiguousarray(np.asarray(ln_b, np.float32))
    tw = float(np.asarray(text_w).reshape(-1)[0])
    aw = float(np.asarray(audio_w).reshape(-1)[0])
    fb = float(np.asarray(fbias).reshape(-1)[0])

    key = (tw, aw, fb)
    if key not in _compiled:
        _compiled[key] = _build(tw, aw, fb)
    nc = _compiled[key]

    in_maps = []
    for i in range(N_CORES):
        sl = slice(i * BPC, (i + 1) * BPC)
        in_maps.append({
            "hid": hs[sl], "aud": ad[sl], "am": am[sl],
            "wt": wt, "wa": wa, "dw": dw, "db": db,
            "lnw": lnw, "lnb": lnb,
        })

    if PROFILE:
        _install_profile_hook()
    res = run_bass_kernel_spmd(nc, in_maps, list(range(N_CORES)),
                               trace=PROFILE)
    LAST_EXEC_NS = res.exec_time_ns

    h0 = np.concatenate([res.results[i]["out0"] for i in range(N_CORES)], 0)
    t_att = np.concatenate([res.results[i]["t_att"] for i in range(N_CORES)], 0)
    f_att = np.concatenate([res.results[i]["f_att"] for i in range(N_CORES)], 0)
    return h0, t_att, f_att
